# revision 4
# baseline (speedup 1.0000x reference)
import sys
sys.path.insert(0, "/opt/trn_rl_repo")
import numpy as np

from concourse import bass, tile, bass_utils, mybir
from concourse.bacc import Bacc

F32 = mybir.dt.float32
I32 = mybir.dt.int32
AL = mybir.AluOpType
AF = mybir.ActivationFunctionType

N = 100_000
E_ORIG = 600_000
E = 2 * E_ORIG
NODE_IN, EDGE_IN = 7, 8
H = 64
NC = 8
NSH = N // NC            # 12500 nodes per core
NBLK = (NSH + 127) // 128  # 98
LAST_NR = NSH - (NBLK - 1) * 128  # 84
NCHO = (E_ORIG // NC + 127) // 128  # 586 flow chunks per core
EO_SH = E_ORIG // NC     # 75000


def _bacc():
    return Bacc("TRN2", target_bir_lowering=False, num_devices=NC)


def build_encoder():
    nc = _bacc()
    xT = nc.dram_tensor("xT", (NODE_IN, NSH), F32, kind="ExternalInput")
    Wenc = nc.dram_tensor("Wenc", (NODE_IN, H), F32, kind="ExternalInput")
    benc = nc.dram_tensor("benc", (128, H), F32, kind="ExternalInput")
    Wcat = nc.dram_tensor("Wcat", (H, 72), F32, kind="ExternalInput")
    ident = nc.dram_tensor("ident", (128, 128), F32, kind="ExternalInput")
    tab_out = nc.dram_tensor("tab_out", (NSH, 72), F32, kind="ExternalOutput")
    with tile.TileContext(nc) as tc:
        with (
            tc.tile_pool(name="w", bufs=1) as wpool,
            tc.tile_pool(name="sb", bufs=3) as pool,
            tc.tile_pool(name="psA", bufs=2, space="PSUM") as psA,
            tc.tile_pool(name="psB", bufs=2, space="PSUM") as psB,
        ):
            xT_sb = wpool.tile((NODE_IN, NSH), F32)
            Wenc_sb = wpool.tile((NODE_IN, H), F32)
            benc_sb = wpool.tile((128, H), F32)
            Wcat_sb = wpool.tile((H, 72), F32)
            id_sb = wpool.tile((128, 128), F32)
            nc.sync.dma_start(out=xT_sb[:], in_=xT[:])
            nc.sync.dma_start(out=Wenc_sb[:], in_=Wenc[:])
            nc.sync.dma_start(out=benc_sb[:], in_=benc[:])
            nc.sync.dma_start(out=Wcat_sb[:], in_=Wcat[:])
            nc.sync.dma_start(out=id_sb[:], in_=ident[:])
            for j in range(NBLK):
                nr = 128 if j < NBLK - 1 else LAST_NR
                lb = j * 128
                hp = psA.tile((128, H), F32, name="hp")
                nc.tensor.matmul(hp[0:nr, :], xT_sb[:, lb:lb + nr], Wenc_sb[:],
                                 start=True, stop=True)
                h_sb = pool.tile((128, H), F32, name="h_sb")
                nc.vector.tensor_add(h_sb[0:nr, :], hp[0:nr, :], benc_sb[0:nr, :])
                nc.scalar.activation(h_sb[0:nr, :], h_sb[0:nr, :], AF.Relu)
                htp = psB.tile((H, 128), F32, name="htp")
                nc.tensor.matmul(htp[:, 0:nr], h_sb[0:nr, :], id_sb[0:nr, 0:nr],
                                 start=True, stop=True)
                ht_sb = pool.tile((H, 128), F32, name="ht_sb")
                nc.vector.tensor_copy(ht_sb[:, 0:nr], htp[:, 0:nr])
                tabp = psA.tile((128, 72), F32, name="tabp")
                nc.tensor.matmul(tabp[0:nr, :], ht_sb[:, 0:nr], Wcat_sb[:],
                                 start=True, stop=True)
                tab_sb = pool.tile((128, 72), F32, name="tab_sb")
                nc.vector.tensor_copy(tab_sb[0:nr, :], tabp[0:nr, :])
                nc.sync.dma_start(out=tab_out[lb:lb + nr, :], in_=tab_sb[0:nr, :])
    nc.finalize()
    return nc


def build_gat(nchb, mode, bp2=0.0):
    """mode: 'table' -> outputs next-layer table; 'press' -> outputs h + pressure."""
    NCH = int(np.sum(nchb))
    NCHB_MAX = int(np.max(nchb))
    nc = _bacc()
    tab = nc.dram_tensor("tab", (N, 72), F32, kind="ExternalInput")
    esrcT = nc.dram_tensor("esrcT", (128, NCH), I32, kind="ExternalInput")
    slotT = nc.dram_tensor("slotT", (128, NCH), F32, kind="ExternalInput")
    validT = nc.dram_tensor("validT", (128, NCH), F32, kind="ExternalInput")
    eaTT = nc.dram_tensor("eaTT", (EDGE_IN, NCH * 128), F32, kind="ExternalInput")
    Wae = nc.dram_tensor("Wae", (EDGE_IN, 4), F32, kind="ExternalInput")
    bgat = nc.dram_tensor("bgat", (128, H), F32, kind="ExternalInput")
    iota = nc.dram_tensor("iota", (128, 128), F32, kind="ExternalInput")
    ident = nc.dram_tensor("ident", (128, 128), F32, kind="ExternalInput")
    core_off = nc.dram_tensor("core_off", (1, 1), I32, kind="ExternalInput")  # unused pad
    if mode == "table":
        Wcat = nc.dram_tensor("Wcat", (H, 72), F32, kind="ExternalInput")
        tabn = nc.dram_tensor("tabn", (NSH, 72), F32, kind="ExternalOutput")
    else:
        Wp1 = nc.dram_tensor("Wp1", (H, H), F32, kind="ExternalInput")
        Wp2 = nc.dram_tensor("Wp2", (H, 1), F32, kind="ExternalInput")
        bp1 = nc.dram_tensor("bp1", (128, H), F32, kind="ExternalInput")
        h_out = nc.dram_tensor("h_out", (NSH, H), F32, kind="ExternalOutput")
        press = nc.dram_tensor("press", (NSH, 1), F32, kind="ExternalOutput")
    # per-core dst-shard base offsets are baked by host via aldbase input:
    aldbase = nc.dram_tensor("aldbase", (NSH, 4), F32, kind="ExternalInput")
    with tile.TileContext(nc) as tc:
        with (
            tc.tile_pool(name="w", bufs=1) as wpool,
            tc.tile_pool(name="gal", bufs=4) as galpool,
            tc.tile_pool(name="s", bufs=3) as spool,
            tc.tile_pool(name="s2", bufs=3) as s2pool,
            tc.tile_pool(name="e", bufs=3) as epool,
            tc.tile_pool(name="exg", bufs=3) as exgpool,
            tc.tile_pool(name="ea", bufs=2) as eapool,
            tc.tile_pool(name="ald", bufs=1) as aldpool,
            tc.tile_pool(name="blk", bufs=2) as bpool,
            tc.tile_pool(name="agg", bufs=2, space="PSUM") as aggps,
            tc.tile_pool(name="s2p", bufs=2 if mode == "table" else 1, space="PSUM") as s2ps,
            tc.tile_pool(name="sump", bufs=2 if mode == "table" else 1, space="PSUM") as sumps,
            tc.tile_pool(name="aux", bufs=1, space="PSUM") as auxps,
        ):
            esrc_sb = wpool.tile((128, NCH), I32)
            slot_sb = wpool.tile((128, NCH), F32)
            valid_sb = wpool.tile((128, NCH), F32)
            Wae_sb = wpool.tile((EDGE_IN, 4), F32)
            bgat_sb = wpool.tile((128, H), F32)
            iota_sb = wpool.tile((128, 128), F32)
            id_sb = wpool.tile((128, 128), F32)
            nc.sync.dma_start(out=esrc_sb[:], in_=esrcT[:])
            nc.sync.dma_start(out=slot_sb[:], in_=slotT[:])
            nc.sync.dma_start(out=valid_sb[:], in_=validT[:])
            nc.sync.dma_start(out=Wae_sb[:], in_=Wae[:])
            nc.sync.dma_start(out=bgat_sb[:], in_=bgat[:])
            nc.sync.dma_start(out=iota_sb[:], in_=iota[:])
            nc.sync.dma_start(out=id_sb[:], in_=ident[:])
            if mode == "table":
                Wcat_sb = wpool.tile((H, 72), F32)
                nc.sync.dma_start(out=Wcat_sb[:], in_=Wcat[:])
            else:
                Wp1_sb = wpool.tile((H, H), F32)
                Wp2_sb = wpool.tile((H, 1), F32)
                bp1_sb = wpool.tile((128, H), F32)
                nc.sync.dma_start(out=Wp1_sb[:], in_=Wp1[:])
                nc.sync.dma_start(out=Wp2_sb[:], in_=Wp2[:])
                nc.sync.dma_start(out=bp1_sb[:], in_=bp1[:])
            aldb = aldpool.tile((128, 4), F32)
            cg = 0
            for b in range(NBLK):
                nr = 128 if b < NBLK - 1 else LAST_NR
                lb = b * 128
                nch = int(nchb[b])
                nc.sync.dma_start(out=aldb[0:nr, :], in_=aldbase[lb:lb + nr, :])
                eab = eapool.tile((EDGE_IN, NCHB_MAX * 128), F32, name="eab")
                nc.sync.dma_start(out=eab[:, 0:nch * 128],
                                  in_=eaTT[:, cg * 128:(cg + nch) * 128])
                aggp = aggps.tile((128, 68), F32, name="aggp")
                for k in range(nch):
                    c = cg + k
                    gal = galpool.tile((128, 72), F32, name="gal")
                    nc.gpsimd.indirect_dma_start(
                        out=gal[:], out_offset=None, in_=tab[:],
                        in_offset=bass.IndirectOffsetOnAxis(ap=esrc_sb[:, c:c + 1], axis=0))
                    S = spool.tile((128, 128), F32, name="S")
                    nc.vector.tensor_tensor(
                        S[:], slot_sb[:, c:c + 1].to_broadcast((128, 128)),
                        iota_sb[:], AL.is_equal)
                    s2p = s2ps.tile((128, 128), F32, name="s2p")
                    nc.tensor.matmul(s2p[:], S[:], id_sb[:], start=True, stop=True)
                    S2 = s2pool.tile((128, 128), F32, name="S2")
                    nc.vector.tensor_copy(S2[:], s2p[:])
                    sump = sumps.tile((128, 4), F32, name="sump")
                    nc.tensor.matmul(sump[:], eab[:, k * 128:(k + 1) * 128], Wae_sb[:],
                                     start=True, stop=False)
                    nc.tensor.matmul(sump[:], S2[:], aldb[:], start=False, stop=True)
                    e_sb = epool.tile((128, 4), F32, name="e_sb")
                    nc.vector.tensor_add(e_sb[:], gal[:, 64:68], sump[:])
                    nc.vector.scalar_tensor_tensor(e_sb[:], e_sb[:], 0.2, e_sb[:],
                                                   AL.mult, AL.max)
                    nc.scalar.activation(e_sb[:], e_sb[:], AF.Exp)
                    nc.vector.tensor_scalar_mul(e_sb[:], e_sb[:], valid_sb[:, c:c + 1])
                    exg = exgpool.tile((128, 68), F32, name="exg")
                    for hh in range(4):
                        nc.vector.tensor_scalar_mul(
                            exg[:, hh * 16:(hh + 1) * 16],
                            gal[:, hh * 16:(hh + 1) * 16], e_sb[:, hh:hh + 1])
                    nc.vector.tensor_copy(exg[:, 64:68], e_sb[:])
                    nc.tensor.matmul(aggp[:], S[:], exg[:],
                                     start=(k == 0), stop=(k == nch - 1))
                cg += nch
                den = bpool.tile((128, 4), F32, name="den")
                nc.vector.tensor_scalar_max(den[:], aggp[:, 64:68], 1e-30)
                denr = bpool.tile((128, 4), F32, name="denr")
                nc.vector.reciprocal(denr[:], den[:])
                h_sb = bpool.tile((128, H), F32, name="h_sb")
                for hh in range(4):
                    nc.vector.tensor_scalar_mul(
                        h_sb[:, hh * 16:(hh + 1) * 16],
                        aggp[:, hh * 16:(hh + 1) * 16], denr[:, hh:hh + 1])
                nc.vector.tensor_add(h_sb[:], h_sb[:], bgat_sb[:])
                nc.scalar.activation(h_sb[:], h_sb[:], AF.Relu)
                htp = auxps.tile((H, 128), F32, name="htp")
                nc.tensor.matmul(htp[:], h_sb[:], id_sb[:], start=True, stop=True)
                ht_sb = bpool.tile((H, 128), F32, name="ht_sb")
                nc.vector.tensor_copy(ht_sb[:], htp[:])
                if mode == "table":
                    tabp = auxps.tile((128, 72), F32, name="tabp")
                    nc.tensor.matmul(tabp[:], ht_sb[:], Wcat_sb[:], start=True, stop=True)
                    tab_sb = bpool.tile((128, 72), F32, name="tab_sb")
                    nc.vector.tensor_copy(tab_sb[:], tabp[:])
                    nc.sync.dma_start(out=tabn[lb:lb + nr, :], in_=tab_sb[0:nr, :])
                else:
                    nc.sync.dma_start(out=h_out[lb:lb + nr, :], in_=h_sb[0:nr, :])
                    zp = auxps.tile((128, H), F32, name="zp")
                    nc.tensor.matmul(zp[:], ht_sb[:], Wp1_sb[:], start=True, stop=True)
                    z_sb = bpool.tile((128, H), F32, name="z_sb")
                    nc.vector.tensor_add(z_sb[:], zp[:], bp1_sb[:])
                    nc.scalar.activation(z_sb[:], z_sb[:], AF.Relu)
                    ztp = auxps.tile((H, 128), F32, name="ztp")
                    nc.tensor.matmul(ztp[:], z_sb[:], id_sb[:], start=True, stop=True)
                    zt_sb = bpool.tile((H, 128), F32, name="zt_sb")
                    nc.vector.tensor_copy(zt_sb[:], ztp[:])
                    pp = auxps.tile((128, 1), F32, name="pp")
                    nc.tensor.matmul(pp[:], zt_sb[:], Wp2_sb[:], start=True, stop=True)
                    p_sb = bpool.tile((128, 1), F32, name="p_sb")
                    nc.vector.tensor_scalar_add(p_sb[:], pp[:], float(bp2))
                    nc.sync.dma_start(out=press[lb:lb + nr, :], in_=p_sb[0:nr, :])
    nc.finalize()
    return nc


def build_flow(bf2=0.0):
    nc = _bacc()
    htab = nc.dram_tensor("htab", (N, H), F32, kind="ExternalInput")
    esrcT = nc.dram_tensor("esrcT", (128, NCHO), I32, kind="ExternalInput")
    edstT = nc.dram_tensor("edstT", (128, NCHO), I32, kind="ExternalInput")
    eaTT = nc.dram_tensor("eaTT", (EDGE_IN, NCHO * 128), F32, kind="ExternalInput")
    Wf1s = nc.dram_tensor("Wf1s", (H, H), F32, kind="ExternalInput")
    Wf1d = nc.dram_tensor("Wf1d", (H, H), F32, kind="ExternalInput")
    Wf1e = nc.dram_tensor("Wf1e", (EDGE_IN, H), F32, kind="ExternalInput")
    Wf2 = nc.dram_tensor("Wf2", (H, 1), F32, kind="ExternalInput")
    bf1 = nc.dram_tensor("bf1", (128, H), F32, kind="ExternalInput")
    ident = nc.dram_tensor("ident", (128, 128), F32, kind="ExternalInput")
    flowT = nc.dram_tensor("flowT", (128, NCHO), F32, kind="ExternalOutput")
    GRP = 64
    with tile.TileContext(nc) as tc:
        with (
            tc.tile_pool(name="w", bufs=1) as wpool,
            tc.tile_pool(name="g", bufs=4) as gpool,
            tc.tile_pool(name="t", bufs=3) as tpool,
            tc.tile_pool(name="ea", bufs=2) as eapool,
            tc.tile_pool(name="psA", bufs=1, space="PSUM") as psA,
            tc.tile_pool(name="psB", bufs=2, space="PSUM") as psB,
            tc.tile_pool(name="psC", bufs=2, space="PSUM") as psC,
        ):
            esrc_sb = wpool.tile((128, NCHO), I32)
            edst_sb = wpool.tile((128, NCHO), I32)
            Wf1s_sb = wpool.tile((H, H), F32)
            Wf1d_sb = wpool.tile((H, H), F32)
            Wf1e_sb = wpool.tile((EDGE_IN, H), F32)
            Wf2_sb = wpool.tile((H, 1), F32)
            bf1_sb = wpool.tile((128, H), F32)
            id_sb = wpool.tile((128, 128), F32)
            flow_sb = wpool.tile((128, NCHO), F32)
            nc.sync.dma_start(out=esrc_sb[:], in_=esrcT[:])
            nc.sync.dma_start(out=edst_sb[:], in_=edstT[:])
            nc.sync.dma_start(out=Wf1s_sb[:], in_=Wf1s[:])
            nc.sync.dma_start(out=Wf1d_sb[:], in_=Wf1d[:])
            nc.sync.dma_start(out=Wf1e_sb[:], in_=Wf1e[:])
            nc.sync.dma_start(out=Wf2_sb[:], in_=Wf2[:])
            nc.sync.dma_start(out=bf1_sb[:], in_=bf1[:])
            nc.sync.dma_start(out=id_sb[:], in_=ident[:])
            for g0 in range(0, NCHO, GRP):
                g1 = min(g0 + GRP, NCHO)
                eab = eapool.tile((EDGE_IN, GRP * 128), F32, name="eab")
                nc.sync.dma_start(out=eab[:, 0:(g1 - g0) * 128],
                                  in_=eaTT[:, g0 * 128:g1 * 128])
                for k in range(g0, g1):
                    kk = k - g0
                    gs = gpool.tile((128, H), F32, name="gs")
                    nc.gpsimd.indirect_dma_start(
                        out=gs[:], out_offset=None, in_=htab[:],
                        in_offset=bass.IndirectOffsetOnAxis(ap=esrc_sb[:, k:k + 1], axis=0))
                    gd = gpool.tile((128, H), F32, name="gd")
                    nc.gpsimd.indirect_dma_start(
                        out=gd[:], out_offset=None, in_=htab[:],
                        in_offset=bass.IndirectOffsetOnAxis(ap=edst_sb[:, k:k + 1], axis=0))
                    gstp = psA.tile((H, 128), F32, name="gstp")
                    nc.tensor.matmul(gstp[:], gs[:], id_sb[:], start=True, stop=True)
                    gst = tpool.tile((H, 128), F32, name="gst")
                    nc.vector.tensor_copy(gst[:], gstp[:])
                    gdtp = psA.tile((H, 128), F32, name="gdtp")
                    nc.tensor.matmul(gdtp[:], gd[:], id_sb[:], start=True, stop=True)
                    gdt = tpool.tile((H, 128), F32, name="gdt")
                    nc.vector.tensor_copy(gdt[:], gdtp[:])
                    zp = psB.tile((128, H), F32, name="zp")
                    nc.tensor.matmul(zp[:], gst[:], Wf1s_sb[:], start=True, stop=False)
                    nc.tensor.matmul(zp[:], gdt[:], Wf1d_sb[:], start=False, stop=False)
                    nc.tensor.matmul(zp[:], eab[:, kk * 128:(kk + 1) * 128], Wf1e_sb[:],
                                     start=False, stop=True)
                    z_sb = tpool.tile((128, H), F32, name="z_sb")
                    nc.vector.tensor_add(z_sb[:], zp[:], bf1_sb[:])
                    nc.scalar.activation(z_sb[:], z_sb[:], AF.Relu)
                    ztp = psA.tile((H, 128), F32, name="ztp")
                    nc.tensor.matmul(ztp[:], z_sb[:], id_sb[:], start=True, stop=True)
                    zt_sb = tpool.tile((H, 128), F32, name="zt_sb")
                    nc.vector.tensor_copy(zt_sb[:], ztp[:])
                    fp = psC.tile((128, 1), F32, name="fp")
                    nc.tensor.matmul(fp[:], zt_sb[:], Wf2_sb[:], start=True, stop=True)
                    nc.vector.tensor_scalar_add(flow_sb[:, k:k + 1], fp[:], float(bf2))
            nc.sync.dma_start(out=flowT[:], in_=flow_sb[:])
    nc.finalize()
    return nc


def _prep_edges(src, dst, edge_attr):
    """dst-sorted, per-core per-block chunked edge arrays (shared by both GAT layers)."""
    order = np.argsort(dst, kind="stable")
    dsts = dst[order]
    srcs = src[order]
    ea_s = edge_attr[order]
    bounds = np.searchsorted(dsts, np.arange(NC + 1) * NSH)
    percore = []
    cnts = np.zeros((NC, NBLK), np.int64)
    for c in range(NC):
        a, bnd = int(bounds[c]), int(bounds[c + 1])
        loc = dsts[a:bnd] - c * NSH
        blk = loc // 128
        cnt = np.bincount(blk, minlength=NBLK)
        starts = np.concatenate([[0], np.cumsum(cnt)])
        percore.append((a, loc, starts))
        cnts[c] = cnt
    nchb = np.maximum((cnts + 127) // 128, 1).max(axis=0)  # [NBLK] shared
    NCH = int(nchb.sum())
    cg0 = np.concatenate([[0], np.cumsum(nchb)])
    esrcT = np.zeros((NC, 128, NCH), np.int32)
    slotT = np.zeros((NC, 128, NCH), np.float32)
    validT = np.zeros((NC, 128, NCH), np.float32)
    eaTT = np.zeros((NC, EDGE_IN, NCH * 128), np.float32)
    for c in range(NC):
        a, loc, starts = percore[c]
        for b in range(NBLK):
            n = int(cnts[c, b])
            if n == 0:
                continue
            s = a + int(starts[b])
            j = np.arange(n)
            ch = int(cg0[b]) + j // 128
            row = j % 128
            esrcT[c, row, ch] = srcs[s:s + n]
            slotT[c, row, ch] = (loc[int(starts[b]):int(starts[b]) + n] - b * 128)
            validT[c, row, ch] = 1.0
            eaTT[c, :, ch * 128 + row] = ea_s[s:s + n]
    return nchb, esrcT, slotT, validT, eaTT


def kernel(x, edge_index, edge_attr, is_original_edge,
           W_enc, b_enc,
           W0, We0, asrc0, adst0, ae0, b0,
           W1, We1, asrc1, adst1, ae1, b1,
           Wp1, bp1, Wp2, bp2, Wf1, bf1, Wf2, bf2):
    x = np.asarray(x, np.float32)
    edge_index = np.asarray(edge_index)
    edge_attr = np.asarray(edge_attr, np.float32)
    src = edge_index[0].astype(np.int64)
    dst = edge_index[1].astype(np.int64)
    f32 = lambda a: np.asarray(a, np.float32)
    W_enc, b_enc = f32(W_enc), f32(b_enc)
    Wp1, bp1, Wp2, bp2 = f32(Wp1), f32(bp1), f32(Wp2), f32(bp2)
    Wf1, bf1, Wf2, bf2 = f32(Wf1), f32(bf1), f32(Wf2), f32(bf2)

    def wcat(W, asrc, adst):
        W, asrc, adst = f32(W), f32(asrc), f32(adst)
        Wm = W.reshape(H, H)
        Ws = np.einsum("dhk,hk->dh", W.reshape(H, 4, 16), asrc)
        Wd = np.einsum("dhk,hk->dh", W.reshape(H, 4, 16), adst)
        return np.ascontiguousarray(np.concatenate([Wm, Ws, Wd], axis=1))

    Wcat0 = wcat(W0, asrc0, adst0)
    Wcat1 = wcat(W1, asrc1, adst1)
    Wae0 = np.ascontiguousarray(np.einsum("dhk,hk->dh", f32(We0).reshape(EDGE_IN, 4, 16), f32(ae0)))
    Wae1 = np.ascontiguousarray(np.einsum("dhk,hk->dh", f32(We1).reshape(EDGE_IN, 4, 16), f32(ae1)))
    ident = np.eye(128, dtype=np.float32)
    iota_r = np.broadcast_to(np.arange(128, dtype=np.float32)[None, :], (128, 128)).copy()
    tile128 = lambda v: np.ascontiguousarray(np.broadcast_to(f32(v).reshape(1, -1), (128, len(np.ravel(v)))))

    nchb, esrcT, slotT, validT, eaTT = _prep_edges(src, dst, edge_attr)

    # ---- launch A: encoder + layer-0 table ----
    ncA = build_encoder()
    xT = np.ascontiguousarray(x.T)
    maps = [{"xT": np.ascontiguousarray(xT[:, c * NSH:(c + 1) * NSH]),
             "Wenc": W_enc, "benc": tile128(b_enc), "Wcat": Wcat0, "ident": ident}
            for c in range(NC)]
    resA = bass_utils.run_bass_kernel_spmd(ncA, maps, list(range(NC)))
    tab0 = np.concatenate([resA.results[c]["tab_out"] for c in range(NC)], axis=0)

    # ---- launch B1: GAT layer 0 -> layer-1 table ----
    ncB1 = build_gat(nchb, "table")
    zoff = np.zeros((1, 1), np.int32)
    maps = [{"tab": tab0, "esrcT": esrcT[c], "slotT": slotT[c], "validT": validT[c],
             "eaTT": eaTT[c], "Wae": Wae0, "bgat": tile128(b0), "iota": iota_r,
             "ident": ident, "core_off": zoff, "Wcat": Wcat1,
             "aldbase": np.ascontiguousarray(tab0[c * NSH:(c + 1) * NSH, 68:72])}
            for c in range(NC)]
    resB1 = bass_utils.run_bass_kernel_spmd(ncB1, maps, list(range(NC)))
    tab1 = np.concatenate([resB1.results[c]["tabn"] for c in range(NC)], axis=0)

    # ---- launch B2: GAT layer 1 -> h2 + pressure ----
    ncB2 = build_gat(nchb, "press", bp2=float(np.ravel(bp2)[0]))
    maps = [{"tab": tab1, "esrcT": esrcT[c], "slotT": slotT[c], "validT": validT[c],
             "eaTT": eaTT[c], "Wae": Wae1, "bgat": tile128(b1), "iota": iota_r,
             "ident": ident, "core_off": zoff,
             "Wp1": Wp1, "Wp2": Wp2.reshape(H, 1), "bp1": tile128(bp1),
             "aldbase": np.ascontiguousarray(tab1[c * NSH:(c + 1) * NSH, 68:72])}
            for c in range(NC)]
    resB2 = bass_utils.run_bass_kernel_spmd(ncB2, maps, list(range(NC)))
    h2 = np.concatenate([resB2.results[c]["h_out"] for c in range(NC)], axis=0)
    pressure = np.concatenate([resB2.results[c]["press"] for c in range(NC)], axis=0)[:, 0]

    # ---- launch C: flow MLP on original edges ----
    oidx = np.nonzero(np.asarray(is_original_edge))[0][:E_ORIG]
    ncC = build_flow(bf2=float(np.ravel(bf2)[0]))
    Wf1s = np.ascontiguousarray(Wf1[0:H, :])
    Wf1d = np.ascontiguousarray(Wf1[H:2 * H, :])
    Wf1e = np.ascontiguousarray(Wf1[2 * H:, :])
    maps = []
    for c in range(NC):
        oi = oidx[c * EO_SH:(c + 1) * EO_SH]
        es = np.zeros((NCHO * 128,), np.int32)
        ed = np.zeros((NCHO * 128,), np.int32)
        es[:EO_SH] = src[oi]
        ed[:EO_SH] = dst[oi]
        eao = np.zeros((NCHO * 128, EDGE_IN), np.float32)
        eao[:EO_SH] = edge_attr[oi]
        maps.append({"htab": h2,
                     "esrcT": np.ascontiguousarray(es.reshape(NCHO, 128).T),
                     "edstT": np.ascontiguousarray(ed.reshape(NCHO, 128).T),
                     "eaTT": np.ascontiguousarray(eao.T),
                     "Wf1s": Wf1s, "Wf1d": Wf1d, "Wf1e": Wf1e,
                     "Wf2": Wf2.reshape(H, 1), "bf1": tile128(bf1), "ident": ident})
    resC = bass_utils.run_bass_kernel_spmd(ncC, maps, list(range(NC)))
    flow = np.concatenate(
        [resC.results[c]["flowT"].T.ravel()[:EO_SH] for c in range(NC)])

    return pressure.astype(np.float32), flow.astype(np.float32), h2.astype(np.float32)


# revision 6
# speedup vs baseline: 1.4851x; 1.4851x over previous
import sys
sys.path.insert(0, "/opt/trn_rl_repo")
import numpy as np

from concourse import bass, tile, bass_utils, mybir
from concourse.bacc import Bacc

F32 = mybir.dt.float32
I32 = mybir.dt.int32
AL = mybir.AluOpType
AF = mybir.ActivationFunctionType

N = 100_000
E_ORIG = 600_000
E = 2 * E_ORIG
NODE_IN, EDGE_IN = 7, 8
H = 64
NC = 8
NSH = N // NC            # 12500 nodes per core
NBLK = (NSH + 127) // 128  # 98
LAST_NR = NSH - (NBLK - 1) * 128  # 84
NCHO = (E_ORIG // NC + 127) // 128  # 586 flow chunks per core
EO_SH = E_ORIG // NC     # 75000


def _bacc():
    return Bacc("TRN2", target_bir_lowering=False, num_devices=NC)


_BUILD_CACHE = {}


def _cached(key, fn):
    if key not in _BUILD_CACHE:
        _BUILD_CACHE[key] = fn()
    return _BUILD_CACHE[key]


def build_encoder():
    nc = _bacc()
    xT = nc.dram_tensor("xT", (NODE_IN, NSH), F32, kind="ExternalInput")
    Wenc = nc.dram_tensor("Wenc", (NODE_IN, H), F32, kind="ExternalInput")
    benc = nc.dram_tensor("benc", (128, H), F32, kind="ExternalInput")
    Wcat = nc.dram_tensor("Wcat", (H, 72), F32, kind="ExternalInput")
    ident = nc.dram_tensor("ident", (128, 128), F32, kind="ExternalInput")
    tab_out = nc.dram_tensor("tab_out", (NSH, 72), F32, kind="ExternalOutput")
    with tile.TileContext(nc) as tc:
        with (
            tc.tile_pool(name="w", bufs=1) as wpool,
            tc.tile_pool(name="sb", bufs=3) as pool,
            tc.tile_pool(name="psA", bufs=2, space="PSUM") as psA,
            tc.tile_pool(name="psB", bufs=2, space="PSUM") as psB,
        ):
            xT_sb = wpool.tile((NODE_IN, NSH), F32)
            Wenc_sb = wpool.tile((NODE_IN, H), F32)
            benc_sb = wpool.tile((128, H), F32)
            Wcat_sb = wpool.tile((H, 72), F32)
            id_sb = wpool.tile((128, 128), F32)
            nc.sync.dma_start(out=xT_sb[:], in_=xT[:])
            nc.sync.dma_start(out=Wenc_sb[:], in_=Wenc[:])
            nc.sync.dma_start(out=benc_sb[:], in_=benc[:])
            nc.sync.dma_start(out=Wcat_sb[:], in_=Wcat[:])
            nc.sync.dma_start(out=id_sb[:], in_=ident[:])
            for j in range(NBLK):
                nr = 128 if j < NBLK - 1 else LAST_NR
                lb = j * 128
                hp = psA.tile((128, H), F32, name="hp")
                nc.tensor.matmul(hp[0:nr, :], xT_sb[:, lb:lb + nr], Wenc_sb[:],
                                 start=True, stop=True)
                h_sb = pool.tile((128, H), F32, name="h_sb")
                nc.vector.tensor_add(h_sb[0:nr, :], hp[0:nr, :], benc_sb[0:nr, :])
                nc.scalar.activation(h_sb[0:nr, :], h_sb[0:nr, :], AF.Relu)
                htp = psB.tile((H, 128), F32, name="htp")
                nc.tensor.matmul(htp[:, 0:nr], h_sb[0:nr, :], id_sb[0:nr, 0:nr],
                                 start=True, stop=True)
                ht_sb = pool.tile((H, 128), F32, name="ht_sb")
                nc.vector.tensor_copy(ht_sb[:, 0:nr], htp[:, 0:nr])
                tabp = psA.tile((128, 72), F32, name="tabp")
                nc.tensor.matmul(tabp[0:nr, :], ht_sb[:, 0:nr], Wcat_sb[:],
                                 start=True, stop=True)
                tab_sb = pool.tile((128, 72), F32, name="tab_sb")
                nc.vector.tensor_copy(tab_sb[0:nr, :], tabp[0:nr, :])
                nc.sync.dma_start(out=tab_out[lb:lb + nr, :], in_=tab_sb[0:nr, :])
    nc.finalize()
    return nc


def build_gat(nchb, mode, bp2=0.0):
    """mode: 'table' -> outputs next-layer table; 'press' -> outputs h + pressure."""
    NCH = int(np.sum(nchb))
    NCHB_MAX = int(np.max(nchb))
    nc = _bacc()
    tab = nc.dram_tensor("tab", (N, 72), F32, kind="ExternalInput")
    esrcT = nc.dram_tensor("esrcT", (128, NCH), I32, kind="ExternalInput")
    slotT = nc.dram_tensor("slotT", (128, NCH), F32, kind="ExternalInput")
    validT = nc.dram_tensor("validT", (128, NCH), F32, kind="ExternalInput")
    eaTT = nc.dram_tensor("eaTT", (EDGE_IN, NCH * 128), F32, kind="ExternalInput")
    Wae = nc.dram_tensor("Wae", (EDGE_IN, 4), F32, kind="ExternalInput")
    bgat = nc.dram_tensor("bgat", (128, H), F32, kind="ExternalInput")
    iota = nc.dram_tensor("iota", (128, 128), F32, kind="ExternalInput")
    ident = nc.dram_tensor("ident", (128, 128), F32, kind="ExternalInput")
    core_off = nc.dram_tensor("core_off", (1, 1), I32, kind="ExternalInput")  # unused pad
    if mode == "table":
        Wcat = nc.dram_tensor("Wcat", (H, 72), F32, kind="ExternalInput")
        tabn = nc.dram_tensor("tabn", (NSH, 72), F32, kind="ExternalOutput")
    else:
        Wp1 = nc.dram_tensor("Wp1", (H, H), F32, kind="ExternalInput")
        Wp2 = nc.dram_tensor("Wp2", (H, 1), F32, kind="ExternalInput")
        bp1 = nc.dram_tensor("bp1", (128, H), F32, kind="ExternalInput")
        h_out = nc.dram_tensor("h_out", (NSH, H), F32, kind="ExternalOutput")
        press = nc.dram_tensor("press", (NSH, 1), F32, kind="ExternalOutput")
    # per-core dst-shard base offsets are baked by host via aldbase input:
    aldbase = nc.dram_tensor("aldbase", (NSH, 4), F32, kind="ExternalInput")
    with tile.TileContext(nc) as tc:
        with (
            tc.tile_pool(name="w", bufs=1) as wpool,
            tc.tile_pool(name="gal", bufs=4) as galpool,
            tc.tile_pool(name="s", bufs=3) as spool,
            tc.tile_pool(name="s2", bufs=3) as s2pool,
            tc.tile_pool(name="e", bufs=3) as epool,
            tc.tile_pool(name="exg", bufs=3) as exgpool,
            tc.tile_pool(name="ea", bufs=2) as eapool,
            tc.tile_pool(name="ald", bufs=1) as aldpool,
            tc.tile_pool(name="blk", bufs=2) as bpool,
            tc.tile_pool(name="agg", bufs=2, space="PSUM") as aggps,
            tc.tile_pool(name="s2p", bufs=2 if mode == "table" else 1, space="PSUM") as s2ps,
            tc.tile_pool(name="sump", bufs=2 if mode == "table" else 1, space="PSUM") as sumps,
            tc.tile_pool(name="aux", bufs=1, space="PSUM") as auxps,
        ):
            esrc_sb = wpool.tile((128, NCH), I32)
            slot_sb = wpool.tile((128, NCH), F32)
            valid_sb = wpool.tile((128, NCH), F32)
            Wae_sb = wpool.tile((EDGE_IN, 4), F32)
            bgat_sb = wpool.tile((128, H), F32)
            iota_sb = wpool.tile((128, 128), F32)
            id_sb = wpool.tile((128, 128), F32)
            nc.sync.dma_start(out=esrc_sb[:], in_=esrcT[:])
            nc.sync.dma_start(out=slot_sb[:], in_=slotT[:])
            nc.sync.dma_start(out=valid_sb[:], in_=validT[:])
            nc.sync.dma_start(out=Wae_sb[:], in_=Wae[:])
            nc.sync.dma_start(out=bgat_sb[:], in_=bgat[:])
            nc.sync.dma_start(out=iota_sb[:], in_=iota[:])
            nc.sync.dma_start(out=id_sb[:], in_=ident[:])
            if mode == "table":
                Wcat_sb = wpool.tile((H, 72), F32)
                nc.sync.dma_start(out=Wcat_sb[:], in_=Wcat[:])
            else:
                Wp1_sb = wpool.tile((H, H), F32)
                Wp2_sb = wpool.tile((H, 1), F32)
                bp1_sb = wpool.tile((128, H), F32)
                nc.sync.dma_start(out=Wp1_sb[:], in_=Wp1[:])
                nc.sync.dma_start(out=Wp2_sb[:], in_=Wp2[:])
                nc.sync.dma_start(out=bp1_sb[:], in_=bp1[:])
            aldb = aldpool.tile((128, 4), F32)
            cg = 0
            for b in range(NBLK):
                nr = 128 if b < NBLK - 1 else LAST_NR
                lb = b * 128
                nch = int(nchb[b])
                nc.sync.dma_start(out=aldb[0:nr, :], in_=aldbase[lb:lb + nr, :])
                eab = eapool.tile((EDGE_IN, NCHB_MAX * 128), F32, name="eab")
                nc.sync.dma_start(out=eab[:, 0:nch * 128],
                                  in_=eaTT[:, cg * 128:(cg + nch) * 128])
                aggp = aggps.tile((128, 68), F32, name="aggp")
                for k in range(nch):
                    c = cg + k
                    gal = galpool.tile((128, 72), F32, name="gal")
                    nc.gpsimd.indirect_dma_start(
                        out=gal[:], out_offset=None, in_=tab[:],
                        in_offset=bass.IndirectOffsetOnAxis(ap=esrc_sb[:, c:c + 1], axis=0))
                    S = spool.tile((128, 128), F32, name="S")
                    nc.vector.tensor_tensor(
                        S[:], slot_sb[:, c:c + 1].to_broadcast((128, 128)),
                        iota_sb[:], AL.is_equal)
                    s2p = s2ps.tile((128, 128), F32, name="s2p")
                    nc.tensor.matmul(s2p[:], S[:], id_sb[:], start=True, stop=True)
                    S2 = s2pool.tile((128, 128), F32, name="S2")
                    nc.vector.tensor_copy(S2[:], s2p[:])
                    sump = sumps.tile((128, 4), F32, name="sump")
                    nc.tensor.matmul(sump[:], eab[:, k * 128:(k + 1) * 128], Wae_sb[:],
                                     start=True, stop=False)
                    nc.tensor.matmul(sump[:], S2[:], aldb[:], start=False, stop=True)
                    e_sb = epool.tile((128, 4), F32, name="e_sb")
                    nc.vector.tensor_add(e_sb[:], gal[:, 64:68], sump[:])
                    nc.vector.scalar_tensor_tensor(e_sb[:], e_sb[:], 0.2, e_sb[:],
                                                   AL.mult, AL.max)
                    nc.scalar.activation(e_sb[:], e_sb[:], AF.Exp)
                    nc.vector.tensor_scalar_mul(e_sb[:], e_sb[:], valid_sb[:, c:c + 1])
                    exg = exgpool.tile((128, 68), F32, name="exg")
                    for hh in range(4):
                        nc.vector.tensor_scalar_mul(
                            exg[:, hh * 16:(hh + 1) * 16],
                            gal[:, hh * 16:(hh + 1) * 16], e_sb[:, hh:hh + 1])
                    nc.vector.tensor_copy(exg[:, 64:68], e_sb[:])
                    nc.tensor.matmul(aggp[:], S[:], exg[:],
                                     start=(k == 0), stop=(k == nch - 1))
                cg += nch
                den = bpool.tile((128, 4), F32, name="den")
                nc.vector.tensor_scalar_max(den[:], aggp[:, 64:68], 1e-30)
                denr = bpool.tile((128, 4), F32, name="denr")
                nc.vector.reciprocal(denr[:], den[:])
                h_sb = bpool.tile((128, H), F32, name="h_sb")
                for hh in range(4):
                    nc.vector.tensor_scalar_mul(
                        h_sb[:, hh * 16:(hh + 1) * 16],
                        aggp[:, hh * 16:(hh + 1) * 16], denr[:, hh:hh + 1])
                nc.vector.tensor_add(h_sb[:], h_sb[:], bgat_sb[:])
                nc.scalar.activation(h_sb[:], h_sb[:], AF.Relu)
                htp = auxps.tile((H, 128), F32, name="htp")
                nc.tensor.matmul(htp[:], h_sb[:], id_sb[:], start=True, stop=True)
                ht_sb = bpool.tile((H, 128), F32, name="ht_sb")
                nc.vector.tensor_copy(ht_sb[:], htp[:])
                if mode == "table":
                    tabp = auxps.tile((128, 72), F32, name="tabp")
                    nc.tensor.matmul(tabp[:], ht_sb[:], Wcat_sb[:], start=True, stop=True)
                    tab_sb = bpool.tile((128, 72), F32, name="tab_sb")
                    nc.vector.tensor_copy(tab_sb[:], tabp[:])
                    nc.sync.dma_start(out=tabn[lb:lb + nr, :], in_=tab_sb[0:nr, :])
                else:
                    nc.sync.dma_start(out=h_out[lb:lb + nr, :], in_=h_sb[0:nr, :])
                    zp = auxps.tile((128, H), F32, name="zp")
                    nc.tensor.matmul(zp[:], ht_sb[:], Wp1_sb[:], start=True, stop=True)
                    z_sb = bpool.tile((128, H), F32, name="z_sb")
                    nc.vector.tensor_add(z_sb[:], zp[:], bp1_sb[:])
                    nc.scalar.activation(z_sb[:], z_sb[:], AF.Relu)
                    ztp = auxps.tile((H, 128), F32, name="ztp")
                    nc.tensor.matmul(ztp[:], z_sb[:], id_sb[:], start=True, stop=True)
                    zt_sb = bpool.tile((H, 128), F32, name="zt_sb")
                    nc.vector.tensor_copy(zt_sb[:], ztp[:])
                    pp = auxps.tile((128, 1), F32, name="pp")
                    nc.tensor.matmul(pp[:], zt_sb[:], Wp2_sb[:], start=True, stop=True)
                    p_sb = bpool.tile((128, 1), F32, name="p_sb")
                    nc.vector.tensor_scalar_add(p_sb[:], pp[:], float(bp2))
                    nc.sync.dma_start(out=press[lb:lb + nr, :], in_=p_sb[0:nr, :])
    nc.finalize()
    return nc


def build_flow(bf2=0.0):
    nc = _bacc()
    htab = nc.dram_tensor("htab", (N, H), F32, kind="ExternalInput")
    esrcT = nc.dram_tensor("esrcT", (128, NCHO), I32, kind="ExternalInput")
    edstT = nc.dram_tensor("edstT", (128, NCHO), I32, kind="ExternalInput")
    eaTT = nc.dram_tensor("eaTT", (EDGE_IN, NCHO * 128), F32, kind="ExternalInput")
    Wf1s = nc.dram_tensor("Wf1s", (H, H), F32, kind="ExternalInput")
    Wf1d = nc.dram_tensor("Wf1d", (H, H), F32, kind="ExternalInput")
    Wf1e = nc.dram_tensor("Wf1e", (EDGE_IN, H), F32, kind="ExternalInput")
    Wf2 = nc.dram_tensor("Wf2", (H, 1), F32, kind="ExternalInput")
    bf1 = nc.dram_tensor("bf1", (128, H), F32, kind="ExternalInput")
    ident = nc.dram_tensor("ident", (128, 128), F32, kind="ExternalInput")
    flowT = nc.dram_tensor("flowT", (128, NCHO), F32, kind="ExternalOutput")
    GRP = 64
    with tile.TileContext(nc) as tc:
        with (
            tc.tile_pool(name="w", bufs=1) as wpool,
            tc.tile_pool(name="g", bufs=4) as gpool,
            tc.tile_pool(name="t", bufs=3) as tpool,
            tc.tile_pool(name="ea", bufs=2) as eapool,
            tc.tile_pool(name="psA", bufs=1, space="PSUM") as psA,
            tc.tile_pool(name="psB", bufs=2, space="PSUM") as psB,
            tc.tile_pool(name="psC", bufs=2, space="PSUM") as psC,
        ):
            esrc_sb = wpool.tile((128, NCHO), I32)
            edst_sb = wpool.tile((128, NCHO), I32)
            Wf1s_sb = wpool.tile((H, H), F32)
            Wf1d_sb = wpool.tile((H, H), F32)
            Wf1e_sb = wpool.tile((EDGE_IN, H), F32)
            Wf2_sb = wpool.tile((H, 1), F32)
            bf1_sb = wpool.tile((128, H), F32)
            id_sb = wpool.tile((128, 128), F32)
            flow_sb = wpool.tile((128, NCHO), F32)
            nc.sync.dma_start(out=esrc_sb[:], in_=esrcT[:])
            nc.sync.dma_start(out=edst_sb[:], in_=edstT[:])
            nc.sync.dma_start(out=Wf1s_sb[:], in_=Wf1s[:])
            nc.sync.dma_start(out=Wf1d_sb[:], in_=Wf1d[:])
            nc.sync.dma_start(out=Wf1e_sb[:], in_=Wf1e[:])
            nc.sync.dma_start(out=Wf2_sb[:], in_=Wf2[:])
            nc.sync.dma_start(out=bf1_sb[:], in_=bf1[:])
            nc.sync.dma_start(out=id_sb[:], in_=ident[:])
            for g0 in range(0, NCHO, GRP):
                g1 = min(g0 + GRP, NCHO)
                eab = eapool.tile((EDGE_IN, GRP * 128), F32, name="eab")
                nc.sync.dma_start(out=eab[:, 0:(g1 - g0) * 128],
                                  in_=eaTT[:, g0 * 128:g1 * 128])
                for k in range(g0, g1):
                    kk = k - g0
                    gs = gpool.tile((128, H), F32, name="gs")
                    nc.gpsimd.indirect_dma_start(
                        out=gs[:], out_offset=None, in_=htab[:],
                        in_offset=bass.IndirectOffsetOnAxis(ap=esrc_sb[:, k:k + 1], axis=0))
                    gd = gpool.tile((128, H), F32, name="gd")
                    nc.gpsimd.indirect_dma_start(
                        out=gd[:], out_offset=None, in_=htab[:],
                        in_offset=bass.IndirectOffsetOnAxis(ap=edst_sb[:, k:k + 1], axis=0))
                    gstp = psA.tile((H, 128), F32, name="gstp")
                    nc.tensor.matmul(gstp[:], gs[:], id_sb[:], start=True, stop=True)
                    gst = tpool.tile((H, 128), F32, name="gst")
                    nc.vector.tensor_copy(gst[:], gstp[:])
                    gdtp = psA.tile((H, 128), F32, name="gdtp")
                    nc.tensor.matmul(gdtp[:], gd[:], id_sb[:], start=True, stop=True)
                    gdt = tpool.tile((H, 128), F32, name="gdt")
                    nc.vector.tensor_copy(gdt[:], gdtp[:])
                    zp = psB.tile((128, H), F32, name="zp")
                    nc.tensor.matmul(zp[:], gst[:], Wf1s_sb[:], start=True, stop=False)
                    nc.tensor.matmul(zp[:], gdt[:], Wf1d_sb[:], start=False, stop=False)
                    nc.tensor.matmul(zp[:], eab[:, kk * 128:(kk + 1) * 128], Wf1e_sb[:],
                                     start=False, stop=True)
                    z_sb = tpool.tile((128, H), F32, name="z_sb")
                    nc.vector.tensor_add(z_sb[:], zp[:], bf1_sb[:])
                    nc.scalar.activation(z_sb[:], z_sb[:], AF.Relu)
                    ztp = psA.tile((H, 128), F32, name="ztp")
                    nc.tensor.matmul(ztp[:], z_sb[:], id_sb[:], start=True, stop=True)
                    zt_sb = tpool.tile((H, 128), F32, name="zt_sb")
                    nc.vector.tensor_copy(zt_sb[:], ztp[:])
                    fp = psC.tile((128, 1), F32, name="fp")
                    nc.tensor.matmul(fp[:], zt_sb[:], Wf2_sb[:], start=True, stop=True)
                    nc.vector.tensor_scalar_add(flow_sb[:, k:k + 1], fp[:], float(bf2))
            nc.sync.dma_start(out=flowT[:], in_=flow_sb[:])
    nc.finalize()
    return nc


def _prep_edges(src, dst, edge_attr):
    """dst-sorted, per-core per-block chunked edge arrays (shared by both GAT layers)."""
    order = np.argsort(dst, kind="stable")
    dsts = dst[order]
    srcs = src[order]
    ea_s = edge_attr[order]
    bounds = np.searchsorted(dsts, np.arange(NC + 1) * NSH)
    percore = []
    cnts = np.zeros((NC, NBLK), np.int64)
    for c in range(NC):
        a, bnd = int(bounds[c]), int(bounds[c + 1])
        loc = dsts[a:bnd] - c * NSH
        blk = loc // 128
        cnt = np.bincount(blk, minlength=NBLK)
        starts = np.concatenate([[0], np.cumsum(cnt)])
        percore.append((a, loc, starts))
        cnts[c] = cnt
    nchb = np.maximum((cnts + 127) // 128, 1).max(axis=0)  # [NBLK] shared
    NCH = int(nchb.sum())
    cg0 = np.concatenate([[0], np.cumsum(nchb)])
    esrcT = np.zeros((NC, 128, NCH), np.int32)
    slotT = np.zeros((NC, 128, NCH), np.float32)
    validT = np.zeros((NC, 128, NCH), np.float32)
    eaTT = np.zeros((NC, EDGE_IN, NCH * 128), np.float32)
    for c in range(NC):
        a, loc, starts = percore[c]
        for b in range(NBLK):
            n = int(cnts[c, b])
            if n == 0:
                continue
            s = a + int(starts[b])
            j = np.arange(n)
            ch = int(cg0[b]) + j // 128
            row = j % 128
            esrcT[c, row, ch] = srcs[s:s + n]
            slotT[c, row, ch] = (loc[int(starts[b]):int(starts[b]) + n] - b * 128)
            validT[c, row, ch] = 1.0
            eaTT[c, :, ch * 128 + row] = ea_s[s:s + n]
    return nchb, esrcT, slotT, validT, eaTT


def kernel(x, edge_index, edge_attr, is_original_edge,
           W_enc, b_enc,
           W0, We0, asrc0, adst0, ae0, b0,
           W1, We1, asrc1, adst1, ae1, b1,
           Wp1, bp1, Wp2, bp2, Wf1, bf1, Wf2, bf2):
    x = np.asarray(x, np.float32)
    edge_index = np.asarray(edge_index)
    edge_attr = np.asarray(edge_attr, np.float32)
    src = edge_index[0].astype(np.int64)
    dst = edge_index[1].astype(np.int64)
    f32 = lambda a: np.asarray(a, np.float32)
    W_enc, b_enc = f32(W_enc), f32(b_enc)
    Wp1, bp1, Wp2, bp2 = f32(Wp1), f32(bp1), f32(Wp2), f32(bp2)
    Wf1, bf1, Wf2, bf2 = f32(Wf1), f32(bf1), f32(Wf2), f32(bf2)

    def wcat(W, asrc, adst):
        W, asrc, adst = f32(W), f32(asrc), f32(adst)
        Wm = W.reshape(H, H)
        Ws = np.einsum("dhk,hk->dh", W.reshape(H, 4, 16), asrc)
        Wd = np.einsum("dhk,hk->dh", W.reshape(H, 4, 16), adst)
        return np.ascontiguousarray(np.concatenate([Wm, Ws, Wd], axis=1))

    Wcat0 = wcat(W0, asrc0, adst0)
    Wcat1 = wcat(W1, asrc1, adst1)
    Wae0 = np.ascontiguousarray(np.einsum("dhk,hk->dh", f32(We0).reshape(EDGE_IN, 4, 16), f32(ae0)))
    Wae1 = np.ascontiguousarray(np.einsum("dhk,hk->dh", f32(We1).reshape(EDGE_IN, 4, 16), f32(ae1)))
    ident = np.eye(128, dtype=np.float32)
    iota_r = np.broadcast_to(np.arange(128, dtype=np.float32)[None, :], (128, 128)).copy()
    tile128 = lambda v: np.ascontiguousarray(np.broadcast_to(f32(v).reshape(1, -1), (128, len(np.ravel(v)))))

    nchb, esrcT, slotT, validT, eaTT = _prep_edges(src, dst, edge_attr)

    # ---- launch A: encoder + layer-0 table ----
    ncA = _cached(("enc",), build_encoder)
    xT = np.ascontiguousarray(x.T)
    maps = [{"xT": np.ascontiguousarray(xT[:, c * NSH:(c + 1) * NSH]),
             "Wenc": W_enc, "benc": tile128(b_enc), "Wcat": Wcat0, "ident": ident}
            for c in range(NC)]
    resA = bass_utils.run_bass_kernel_spmd(ncA, maps, list(range(NC)))
    tab0 = np.concatenate([resA.results[c]["tab_out"] for c in range(NC)], axis=0)

    # ---- launch B1: GAT layer 0 -> layer-1 table ----
    ncB1 = _cached(("gat", "table", tuple(nchb)), lambda: build_gat(nchb, "table"))
    zoff = np.zeros((1, 1), np.int32)
    maps = [{"tab": tab0, "esrcT": esrcT[c], "slotT": slotT[c], "validT": validT[c],
             "eaTT": eaTT[c], "Wae": Wae0, "bgat": tile128(b0), "iota": iota_r,
             "ident": ident, "core_off": zoff, "Wcat": Wcat1,
             "aldbase": np.ascontiguousarray(tab0[c * NSH:(c + 1) * NSH, 68:72])}
            for c in range(NC)]
    resB1 = bass_utils.run_bass_kernel_spmd(ncB1, maps, list(range(NC)))
    tab1 = np.concatenate([resB1.results[c]["tabn"] for c in range(NC)], axis=0)

    # ---- launch B2: GAT layer 1 -> h2 + pressure ----
    ncB2 = _cached(("gat", "press", float(np.ravel(bp2)[0]), tuple(nchb)),
                   lambda: build_gat(nchb, "press", bp2=float(np.ravel(bp2)[0])))
    maps = [{"tab": tab1, "esrcT": esrcT[c], "slotT": slotT[c], "validT": validT[c],
             "eaTT": eaTT[c], "Wae": Wae1, "bgat": tile128(b1), "iota": iota_r,
             "ident": ident, "core_off": zoff,
             "Wp1": Wp1, "Wp2": Wp2.reshape(H, 1), "bp1": tile128(bp1),
             "aldbase": np.ascontiguousarray(tab1[c * NSH:(c + 1) * NSH, 68:72])}
            for c in range(NC)]
    resB2 = bass_utils.run_bass_kernel_spmd(ncB2, maps, list(range(NC)))
    h2 = np.concatenate([resB2.results[c]["h_out"] for c in range(NC)], axis=0)
    pressure = np.concatenate([resB2.results[c]["press"] for c in range(NC)], axis=0)[:, 0]

    # ---- launch C: flow MLP on original edges ----
    oidx = np.nonzero(np.asarray(is_original_edge))[0][:E_ORIG]
    ncC = _cached(("flow", float(np.ravel(bf2)[0])),
                  lambda: build_flow(bf2=float(np.ravel(bf2)[0])))
    Wf1s = np.ascontiguousarray(Wf1[0:H, :])
    Wf1d = np.ascontiguousarray(Wf1[H:2 * H, :])
    Wf1e = np.ascontiguousarray(Wf1[2 * H:, :])
    maps = []
    for c in range(NC):
        oi = oidx[c * EO_SH:(c + 1) * EO_SH]
        es = np.zeros((NCHO * 128,), np.int32)
        ed = np.zeros((NCHO * 128,), np.int32)
        es[:EO_SH] = src[oi]
        ed[:EO_SH] = dst[oi]
        eao = np.zeros((NCHO * 128, EDGE_IN), np.float32)
        eao[:EO_SH] = edge_attr[oi]
        maps.append({"htab": h2,
                     "esrcT": np.ascontiguousarray(es.reshape(NCHO, 128).T),
                     "edstT": np.ascontiguousarray(ed.reshape(NCHO, 128).T),
                     "eaTT": np.ascontiguousarray(eao.T),
                     "Wf1s": Wf1s, "Wf1d": Wf1d, "Wf1e": Wf1e,
                     "Wf2": Wf2.reshape(H, 1), "bf1": tile128(bf1), "ident": ident})
    resC = bass_utils.run_bass_kernel_spmd(ncC, maps, list(range(NC)))
    flow = np.concatenate(
        [resC.results[c]["flowT"].T.ravel()[:EO_SH] for c in range(NC)])

    return pressure.astype(np.float32), flow.astype(np.float32), h2.astype(np.float32)


# revision 8
# speedup vs baseline: 1.9377x; 1.3047x over previous
import sys
sys.path.insert(0, "/opt/trn_rl_repo")
import numpy as np

from concourse import bass, tile, bass_utils, mybir
from concourse.bacc import Bacc

F32 = mybir.dt.float32
I32 = mybir.dt.int32
AL = mybir.AluOpType
AF = mybir.ActivationFunctionType

N = 100_000
E_ORIG = 600_000
E = 2 * E_ORIG
NODE_IN, EDGE_IN = 7, 8
H = 64
NC = 8
NSH = N // NC            # 12500 nodes per core
NBLK = (NSH + 127) // 128  # 98
LAST_NR = NSH - (NBLK - 1) * 128  # 84
NCHO = (E_ORIG // NC + 127) // 128  # 586 flow chunks per core
EO_SH = E_ORIG // NC     # 75000


def _bacc():
    return Bacc("TRN2", target_bir_lowering=False, num_devices=NC)


_BUILD_CACHE = {}


def _cached(key, fn):
    if key not in _BUILD_CACHE:
        _BUILD_CACHE[key] = fn()
    return _BUILD_CACHE[key]


_EXEC_CACHE = {}


def _run_spmd(nc, in_maps):
    """run_bass_via_pjrt with the jitted shard_map cached per program."""
    try:
        import jax
        from concourse import bass2jax as b2j
        from jax.experimental.shard_map import shard_map
        from jax.sharding import Mesh, PartitionSpec

        key = id(nc)
        if key not in _EXEC_CACHE:
            b2j.install_neuronx_cc_hook()
            in_names, out_names, out_avals = [], [], []
            partition_name = (nc.partition_id_tensor.name
                              if nc.partition_id_tensor else None)
            for alloc in nc.m.functions[0].allocations:
                if not isinstance(alloc, mybir.MemoryLocationSet):
                    continue
                name = alloc.memorylocations[0].name
                if alloc.kind == "ExternalInput":
                    if name != partition_name:
                        in_names.append(name)
                elif alloc.kind == "ExternalOutput":
                    out_names.append(name)
                    out_avals.append(jax.core.ShapedArray(
                        tuple(alloc.tensor_shape), mybir.dt.np(alloc.dtype)))
            n_params = len(in_names)
            all_names = in_names + out_names
            donate = tuple(range(n_params, n_params + len(out_names)))

            def _body(*args):
                operands = list(args)
                if partition_name is not None:
                    operands.append(b2j.partition_id_tensor())
                return tuple(b2j._bass_exec_p.bind(
                    *operands,
                    out_avals=tuple(out_avals),
                    in_names=tuple(all_names + ([partition_name] if partition_name else [])),
                    out_names=tuple(out_names),
                    lowering_input_output_aliases=(),
                    sim_require_finite=True,
                    sim_require_nnan=True,
                    nc=nc,
                ))

            devices = jax.devices()[:NC]
            mesh = Mesh(np.asarray(devices), ("core",))
            specs = (PartitionSpec("core"),)
            sharded = jax.jit(
                shard_map(_body, mesh=mesh,
                          in_specs=specs * (n_params + len(out_names)),
                          out_specs=specs * len(out_names), check_rep=False),
                donate_argnums=donate, keep_unused=True)
            _EXEC_CACHE[key] = (sharded, in_names, out_names, out_avals, n_params)
        sharded, in_names, out_names, out_avals, n_params = _EXEC_CACHE[key]
        if nc.dbg_addr is not None and nc.dbg_addr.name in in_names:
            in_maps = [{**m, nc.dbg_addr.name: np.zeros((1, 2), np.uint32)}
                       for m in in_maps]
        concat_in = [np.concatenate([np.asarray(m[name]) for m in in_maps], axis=0)
                     for name in in_names]
        concat_zeros = [np.zeros((NC * a.shape[0], *a.shape[1:]), a.dtype)
                        for a in out_avals]
        out_arrs = sharded(*concat_in, *concat_zeros)
        results = []
        for c in range(NC):
            results.append({name: np.asarray(out_arrs[i]).reshape(
                NC, *out_avals[i].shape)[c] for i, name in enumerate(out_names)})
        return results
    except Exception:
        return bass_utils.run_bass_kernel_spmd(nc, in_maps, list(range(NC))).results


def build_encoder():
    nc = _bacc()
    xT = nc.dram_tensor("xT", (NODE_IN, NSH), F32, kind="ExternalInput")
    Wenc = nc.dram_tensor("Wenc", (NODE_IN, H), F32, kind="ExternalInput")
    benc = nc.dram_tensor("benc", (128, H), F32, kind="ExternalInput")
    Wcat = nc.dram_tensor("Wcat", (H, 72), F32, kind="ExternalInput")
    ident = nc.dram_tensor("ident", (128, 128), F32, kind="ExternalInput")
    tab_out = nc.dram_tensor("tab_out", (NSH, 72), F32, kind="ExternalOutput")
    with tile.TileContext(nc) as tc:
        with (
            tc.tile_pool(name="w", bufs=1) as wpool,
            tc.tile_pool(name="sb", bufs=3) as pool,
            tc.tile_pool(name="psA", bufs=2, space="PSUM") as psA,
            tc.tile_pool(name="psB", bufs=2, space="PSUM") as psB,
        ):
            xT_sb = wpool.tile((NODE_IN, NSH), F32)
            Wenc_sb = wpool.tile((NODE_IN, H), F32)
            benc_sb = wpool.tile((128, H), F32)
            Wcat_sb = wpool.tile((H, 72), F32)
            id_sb = wpool.tile((128, 128), F32)
            nc.sync.dma_start(out=xT_sb[:], in_=xT[:])
            nc.sync.dma_start(out=Wenc_sb[:], in_=Wenc[:])
            nc.sync.dma_start(out=benc_sb[:], in_=benc[:])
            nc.sync.dma_start(out=Wcat_sb[:], in_=Wcat[:])
            nc.sync.dma_start(out=id_sb[:], in_=ident[:])
            for j in range(NBLK):
                nr = 128 if j < NBLK - 1 else LAST_NR
                lb = j * 128
                hp = psA.tile((128, H), F32, name="hp")
                nc.tensor.matmul(hp[0:nr, :], xT_sb[:, lb:lb + nr], Wenc_sb[:],
                                 start=True, stop=True)
                h_sb = pool.tile((128, H), F32, name="h_sb")
                nc.vector.tensor_add(h_sb[0:nr, :], hp[0:nr, :], benc_sb[0:nr, :])
                nc.scalar.activation(h_sb[0:nr, :], h_sb[0:nr, :], AF.Relu)
                htp = psB.tile((H, 128), F32, name="htp")
                nc.tensor.matmul(htp[:, 0:nr], h_sb[0:nr, :], id_sb[0:nr, 0:nr],
                                 start=True, stop=True)
                ht_sb = pool.tile((H, 128), F32, name="ht_sb")
                nc.vector.tensor_copy(ht_sb[:, 0:nr], htp[:, 0:nr])
                tabp = psA.tile((128, 72), F32, name="tabp")
                nc.tensor.matmul(tabp[0:nr, :], ht_sb[:, 0:nr], Wcat_sb[:],
                                 start=True, stop=True)
                tab_sb = pool.tile((128, 72), F32, name="tab_sb")
                nc.vector.tensor_copy(tab_sb[0:nr, :], tabp[0:nr, :])
                nc.sync.dma_start(out=tab_out[lb:lb + nr, :], in_=tab_sb[0:nr, :])
    nc.finalize()
    return nc


def build_gat(nchb, mode, bp2=0.0):
    """mode: 'table' -> outputs next-layer table; 'press' -> outputs h + pressure."""
    NCH = int(np.sum(nchb))
    NCHB_MAX = int(np.max(nchb))
    nc = _bacc()
    tab = nc.dram_tensor("tab", (N, 72), F32, kind="ExternalInput")
    esrcT = nc.dram_tensor("esrcT", (128, NCH), I32, kind="ExternalInput")
    slotT = nc.dram_tensor("slotT", (128, NCH), F32, kind="ExternalInput")
    validT = nc.dram_tensor("validT", (128, NCH), F32, kind="ExternalInput")
    eaTT = nc.dram_tensor("eaTT", (EDGE_IN, NCH * 128), F32, kind="ExternalInput")
    Wae = nc.dram_tensor("Wae", (EDGE_IN, 4), F32, kind="ExternalInput")
    bgat = nc.dram_tensor("bgat", (128, H), F32, kind="ExternalInput")
    iota = nc.dram_tensor("iota", (128, 128), F32, kind="ExternalInput")
    ident = nc.dram_tensor("ident", (128, 128), F32, kind="ExternalInput")
    core_off = nc.dram_tensor("core_off", (1, 1), I32, kind="ExternalInput")  # unused pad
    if mode == "table":
        Wcat = nc.dram_tensor("Wcat", (H, 72), F32, kind="ExternalInput")
        tabn = nc.dram_tensor("tabn", (NSH, 72), F32, kind="ExternalOutput")
    else:
        Wp1 = nc.dram_tensor("Wp1", (H, H), F32, kind="ExternalInput")
        Wp2 = nc.dram_tensor("Wp2", (H, 1), F32, kind="ExternalInput")
        bp1 = nc.dram_tensor("bp1", (128, H), F32, kind="ExternalInput")
        h_out = nc.dram_tensor("h_out", (NSH, H), F32, kind="ExternalOutput")
        press = nc.dram_tensor("press", (NSH, 1), F32, kind="ExternalOutput")
    # per-core dst-shard base offsets are baked by host via aldbase input:
    aldbase = nc.dram_tensor("aldbase", (NSH, 4), F32, kind="ExternalInput")
    with tile.TileContext(nc) as tc:
        with (
            tc.tile_pool(name="w", bufs=1) as wpool,
            tc.tile_pool(name="gal", bufs=4) as galpool,
            tc.tile_pool(name="s", bufs=3) as spool,
            tc.tile_pool(name="s2", bufs=3) as s2pool,
            tc.tile_pool(name="e", bufs=3) as epool,
            tc.tile_pool(name="exg", bufs=3) as exgpool,
            tc.tile_pool(name="ea", bufs=2) as eapool,
            tc.tile_pool(name="ald", bufs=1) as aldpool,
            tc.tile_pool(name="blk", bufs=2) as bpool,
            tc.tile_pool(name="agg", bufs=2, space="PSUM") as aggps,
            tc.tile_pool(name="s2p", bufs=2 if mode == "table" else 1, space="PSUM") as s2ps,
            tc.tile_pool(name="sump", bufs=2 if mode == "table" else 1, space="PSUM") as sumps,
            tc.tile_pool(name="aux", bufs=1, space="PSUM") as auxps,
        ):
            esrc_sb = wpool.tile((128, NCH), I32)
            slot_sb = wpool.tile((128, NCH), F32)
            valid_sb = wpool.tile((128, NCH), F32)
            Wae_sb = wpool.tile((EDGE_IN, 4), F32)
            bgat_sb = wpool.tile((128, H), F32)
            iota_sb = wpool.tile((128, 128), F32)
            id_sb = wpool.tile((128, 128), F32)
            nc.sync.dma_start(out=esrc_sb[:], in_=esrcT[:])
            nc.sync.dma_start(out=slot_sb[:], in_=slotT[:])
            nc.sync.dma_start(out=valid_sb[:], in_=validT[:])
            nc.sync.dma_start(out=Wae_sb[:], in_=Wae[:])
            nc.sync.dma_start(out=bgat_sb[:], in_=bgat[:])
            nc.sync.dma_start(out=iota_sb[:], in_=iota[:])
            nc.sync.dma_start(out=id_sb[:], in_=ident[:])
            if mode == "table":
                Wcat_sb = wpool.tile((H, 72), F32)
                nc.sync.dma_start(out=Wcat_sb[:], in_=Wcat[:])
            else:
                Wp1_sb = wpool.tile((H, H), F32)
                Wp2_sb = wpool.tile((H, 1), F32)
                bp1_sb = wpool.tile((128, H), F32)
                nc.sync.dma_start(out=Wp1_sb[:], in_=Wp1[:])
                nc.sync.dma_start(out=Wp2_sb[:], in_=Wp2[:])
                nc.sync.dma_start(out=bp1_sb[:], in_=bp1[:])
            aldb = aldpool.tile((128, 4), F32)
            cg = 0
            for b in range(NBLK):
                nr = 128 if b < NBLK - 1 else LAST_NR
                lb = b * 128
                nch = int(nchb[b])
                nc.sync.dma_start(out=aldb[0:nr, :], in_=aldbase[lb:lb + nr, :])
                eab = eapool.tile((EDGE_IN, NCHB_MAX * 128), F32, name="eab")
                nc.sync.dma_start(out=eab[:, 0:nch * 128],
                                  in_=eaTT[:, cg * 128:(cg + nch) * 128])
                aggp = aggps.tile((128, 68), F32, name="aggp")
                for k in range(nch):
                    c = cg + k
                    gal = galpool.tile((128, 72), F32, name="gal")
                    nc.gpsimd.indirect_dma_start(
                        out=gal[:], out_offset=None, in_=tab[:],
                        in_offset=bass.IndirectOffsetOnAxis(ap=esrc_sb[:, c:c + 1], axis=0))
                    S = spool.tile((128, 128), F32, name="S")
                    nc.vector.tensor_tensor(
                        S[:], slot_sb[:, c:c + 1].to_broadcast((128, 128)),
                        iota_sb[:], AL.is_equal)
                    s2p = s2ps.tile((128, 128), F32, name="s2p")
                    nc.tensor.matmul(s2p[:], S[:], id_sb[:], start=True, stop=True)
                    S2 = s2pool.tile((128, 128), F32, name="S2")
                    nc.vector.tensor_copy(S2[:], s2p[:])
                    sump = sumps.tile((128, 4), F32, name="sump")
                    nc.tensor.matmul(sump[:], eab[:, k * 128:(k + 1) * 128], Wae_sb[:],
                                     start=True, stop=False)
                    nc.tensor.matmul(sump[:], S2[:], aldb[:], start=False, stop=True)
                    e_sb = epool.tile((128, 4), F32, name="e_sb")
                    nc.vector.tensor_add(e_sb[:], gal[:, 64:68], sump[:])
                    nc.vector.scalar_tensor_tensor(e_sb[:], e_sb[:], 0.2, e_sb[:],
                                                   AL.mult, AL.max)
                    nc.scalar.activation(e_sb[:], e_sb[:], AF.Exp)
                    nc.vector.tensor_scalar_mul(e_sb[:], e_sb[:], valid_sb[:, c:c + 1])
                    exg = exgpool.tile((128, 68), F32, name="exg")
                    for hh in range(4):
                        nc.vector.tensor_scalar_mul(
                            exg[:, hh * 16:(hh + 1) * 16],
                            gal[:, hh * 16:(hh + 1) * 16], e_sb[:, hh:hh + 1])
                    nc.vector.tensor_copy(exg[:, 64:68], e_sb[:])
                    nc.tensor.matmul(aggp[:], S[:], exg[:],
                                     start=(k == 0), stop=(k == nch - 1))
                cg += nch
                den = bpool.tile((128, 4), F32, name="den")
                nc.vector.tensor_scalar_max(den[:], aggp[:, 64:68], 1e-30)
                denr = bpool.tile((128, 4), F32, name="denr")
                nc.vector.reciprocal(denr[:], den[:])
                h_sb = bpool.tile((128, H), F32, name="h_sb")
                for hh in range(4):
                    nc.vector.tensor_scalar_mul(
                        h_sb[:, hh * 16:(hh + 1) * 16],
                        aggp[:, hh * 16:(hh + 1) * 16], denr[:, hh:hh + 1])
                nc.vector.tensor_add(h_sb[:], h_sb[:], bgat_sb[:])
                nc.scalar.activation(h_sb[:], h_sb[:], AF.Relu)
                htp = auxps.tile((H, 128), F32, name="htp")
                nc.tensor.matmul(htp[:], h_sb[:], id_sb[:], start=True, stop=True)
                ht_sb = bpool.tile((H, 128), F32, name="ht_sb")
                nc.vector.tensor_copy(ht_sb[:], htp[:])
                if mode == "table":
                    tabp = auxps.tile((128, 72), F32, name="tabp")
                    nc.tensor.matmul(tabp[:], ht_sb[:], Wcat_sb[:], start=True, stop=True)
                    tab_sb = bpool.tile((128, 72), F32, name="tab_sb")
                    nc.vector.tensor_copy(tab_sb[:], tabp[:])
                    nc.sync.dma_start(out=tabn[lb:lb + nr, :], in_=tab_sb[0:nr, :])
                else:
                    nc.sync.dma_start(out=h_out[lb:lb + nr, :], in_=h_sb[0:nr, :])
                    zp = auxps.tile((128, H), F32, name="zp")
                    nc.tensor.matmul(zp[:], ht_sb[:], Wp1_sb[:], start=True, stop=True)
                    z_sb = bpool.tile((128, H), F32, name="z_sb")
                    nc.vector.tensor_add(z_sb[:], zp[:], bp1_sb[:])
                    nc.scalar.activation(z_sb[:], z_sb[:], AF.Relu)
                    ztp = auxps.tile((H, 128), F32, name="ztp")
                    nc.tensor.matmul(ztp[:], z_sb[:], id_sb[:], start=True, stop=True)
                    zt_sb = bpool.tile((H, 128), F32, name="zt_sb")
                    nc.vector.tensor_copy(zt_sb[:], ztp[:])
                    pp = auxps.tile((128, 1), F32, name="pp")
                    nc.tensor.matmul(pp[:], zt_sb[:], Wp2_sb[:], start=True, stop=True)
                    p_sb = bpool.tile((128, 1), F32, name="p_sb")
                    nc.vector.tensor_scalar_add(p_sb[:], pp[:], float(bp2))
                    nc.sync.dma_start(out=press[lb:lb + nr, :], in_=p_sb[0:nr, :])
    nc.finalize()
    return nc


def build_flow(bf2=0.0):
    nc = _bacc()
    htab = nc.dram_tensor("htab", (N, H), F32, kind="ExternalInput")
    esrcT = nc.dram_tensor("esrcT", (128, NCHO), I32, kind="ExternalInput")
    edstT = nc.dram_tensor("edstT", (128, NCHO), I32, kind="ExternalInput")
    eaTT = nc.dram_tensor("eaTT", (EDGE_IN, NCHO * 128), F32, kind="ExternalInput")
    Wf1s = nc.dram_tensor("Wf1s", (H, H), F32, kind="ExternalInput")
    Wf1d = nc.dram_tensor("Wf1d", (H, H), F32, kind="ExternalInput")
    Wf1e = nc.dram_tensor("Wf1e", (EDGE_IN, H), F32, kind="ExternalInput")
    Wf2 = nc.dram_tensor("Wf2", (H, 1), F32, kind="ExternalInput")
    bf1 = nc.dram_tensor("bf1", (128, H), F32, kind="ExternalInput")
    ident = nc.dram_tensor("ident", (128, 128), F32, kind="ExternalInput")
    flowT = nc.dram_tensor("flowT", (128, NCHO), F32, kind="ExternalOutput")
    GRP = 64
    with tile.TileContext(nc) as tc:
        with (
            tc.tile_pool(name="w", bufs=1) as wpool,
            tc.tile_pool(name="g", bufs=4) as gpool,
            tc.tile_pool(name="t", bufs=3) as tpool,
            tc.tile_pool(name="ea", bufs=2) as eapool,
            tc.tile_pool(name="psA", bufs=1, space="PSUM") as psA,
            tc.tile_pool(name="psB", bufs=2, space="PSUM") as psB,
            tc.tile_pool(name="psC", bufs=2, space="PSUM") as psC,
        ):
            esrc_sb = wpool.tile((128, NCHO), I32)
            edst_sb = wpool.tile((128, NCHO), I32)
            Wf1s_sb = wpool.tile((H, H), F32)
            Wf1d_sb = wpool.tile((H, H), F32)
            Wf1e_sb = wpool.tile((EDGE_IN, H), F32)
            Wf2_sb = wpool.tile((H, 1), F32)
            bf1_sb = wpool.tile((128, H), F32)
            id_sb = wpool.tile((128, 128), F32)
            flow_sb = wpool.tile((128, NCHO), F32)
            nc.sync.dma_start(out=esrc_sb[:], in_=esrcT[:])
            nc.sync.dma_start(out=edst_sb[:], in_=edstT[:])
            nc.sync.dma_start(out=Wf1s_sb[:], in_=Wf1s[:])
            nc.sync.dma_start(out=Wf1d_sb[:], in_=Wf1d[:])
            nc.sync.dma_start(out=Wf1e_sb[:], in_=Wf1e[:])
            nc.sync.dma_start(out=Wf2_sb[:], in_=Wf2[:])
            nc.sync.dma_start(out=bf1_sb[:], in_=bf1[:])
            nc.sync.dma_start(out=id_sb[:], in_=ident[:])
            for g0 in range(0, NCHO, GRP):
                g1 = min(g0 + GRP, NCHO)
                eab = eapool.tile((EDGE_IN, GRP * 128), F32, name="eab")
                nc.sync.dma_start(out=eab[:, 0:(g1 - g0) * 128],
                                  in_=eaTT[:, g0 * 128:g1 * 128])
                for k in range(g0, g1):
                    kk = k - g0
                    gs = gpool.tile((128, H), F32, name="gs")
                    nc.gpsimd.indirect_dma_start(
                        out=gs[:], out_offset=None, in_=htab[:],
                        in_offset=bass.IndirectOffsetOnAxis(ap=esrc_sb[:, k:k + 1], axis=0))
                    gd = gpool.tile((128, H), F32, name="gd")
                    nc.gpsimd.indirect_dma_start(
                        out=gd[:], out_offset=None, in_=htab[:],
                        in_offset=bass.IndirectOffsetOnAxis(ap=edst_sb[:, k:k + 1], axis=0))
                    gstp = psA.tile((H, 128), F32, name="gstp")
                    nc.tensor.matmul(gstp[:], gs[:], id_sb[:], start=True, stop=True)
                    gst = tpool.tile((H, 128), F32, name="gst")
                    nc.vector.tensor_copy(gst[:], gstp[:])
                    gdtp = psA.tile((H, 128), F32, name="gdtp")
                    nc.tensor.matmul(gdtp[:], gd[:], id_sb[:], start=True, stop=True)
                    gdt = tpool.tile((H, 128), F32, name="gdt")
                    nc.vector.tensor_copy(gdt[:], gdtp[:])
                    zp = psB.tile((128, H), F32, name="zp")
                    nc.tensor.matmul(zp[:], gst[:], Wf1s_sb[:], start=True, stop=False)
                    nc.tensor.matmul(zp[:], gdt[:], Wf1d_sb[:], start=False, stop=False)
                    nc.tensor.matmul(zp[:], eab[:, kk * 128:(kk + 1) * 128], Wf1e_sb[:],
                                     start=False, stop=True)
                    z_sb = tpool.tile((128, H), F32, name="z_sb")
                    nc.vector.tensor_add(z_sb[:], zp[:], bf1_sb[:])
                    nc.scalar.activation(z_sb[:], z_sb[:], AF.Relu)
                    ztp = psA.tile((H, 128), F32, name="ztp")
                    nc.tensor.matmul(ztp[:], z_sb[:], id_sb[:], start=True, stop=True)
                    zt_sb = tpool.tile((H, 128), F32, name="zt_sb")
                    nc.vector.tensor_copy(zt_sb[:], ztp[:])
                    fp = psC.tile((128, 1), F32, name="fp")
                    nc.tensor.matmul(fp[:], zt_sb[:], Wf2_sb[:], start=True, stop=True)
                    nc.vector.tensor_scalar_add(flow_sb[:, k:k + 1], fp[:], float(bf2))
            nc.sync.dma_start(out=flowT[:], in_=flow_sb[:])
    nc.finalize()
    return nc


def _prep_edges(src, dst, edge_attr):
    """dst-sorted, per-core per-block chunked edge arrays (shared by both GAT layers)."""
    order = np.argsort(dst, kind="stable")
    dsts = dst[order]
    srcs = src[order]
    ea_s = edge_attr[order]
    bounds = np.searchsorted(dsts, np.arange(NC + 1) * NSH)
    percore = []
    cnts = np.zeros((NC, NBLK), np.int64)
    for c in range(NC):
        a, bnd = int(bounds[c]), int(bounds[c + 1])
        loc = dsts[a:bnd] - c * NSH
        blk = loc // 128
        cnt = np.bincount(blk, minlength=NBLK)
        starts = np.concatenate([[0], np.cumsum(cnt)])
        percore.append((a, loc, starts))
        cnts[c] = cnt
    nchb = np.maximum((cnts + 127) // 128, 1).max(axis=0)  # [NBLK] shared
    NCH = int(nchb.sum())
    cg0 = np.concatenate([[0], np.cumsum(nchb)])
    esrcT = np.zeros((NC, 128, NCH), np.int32)
    slotT = np.zeros((NC, 128, NCH), np.float32)
    validT = np.zeros((NC, 128, NCH), np.float32)
    eaTT = np.zeros((NC, EDGE_IN, NCH * 128), np.float32)
    for c in range(NC):
        a, loc, starts = percore[c]
        for b in range(NBLK):
            n = int(cnts[c, b])
            if n == 0:
                continue
            s = a + int(starts[b])
            j = np.arange(n)
            ch = int(cg0[b]) + j // 128
            row = j % 128
            esrcT[c, row, ch] = srcs[s:s + n]
            slotT[c, row, ch] = (loc[int(starts[b]):int(starts[b]) + n] - b * 128)
            validT[c, row, ch] = 1.0
            eaTT[c, :, ch * 128 + row] = ea_s[s:s + n]
    return nchb, esrcT, slotT, validT, eaTT


def kernel(x, edge_index, edge_attr, is_original_edge,
           W_enc, b_enc,
           W0, We0, asrc0, adst0, ae0, b0,
           W1, We1, asrc1, adst1, ae1, b1,
           Wp1, bp1, Wp2, bp2, Wf1, bf1, Wf2, bf2):
    x = np.asarray(x, np.float32)
    edge_index = np.asarray(edge_index)
    edge_attr = np.asarray(edge_attr, np.float32)
    src = edge_index[0].astype(np.int64)
    dst = edge_index[1].astype(np.int64)
    f32 = lambda a: np.asarray(a, np.float32)
    W_enc, b_enc = f32(W_enc), f32(b_enc)
    Wp1, bp1, Wp2, bp2 = f32(Wp1), f32(bp1), f32(Wp2), f32(bp2)
    Wf1, bf1, Wf2, bf2 = f32(Wf1), f32(bf1), f32(Wf2), f32(bf2)

    def wcat(W, asrc, adst):
        W, asrc, adst = f32(W), f32(asrc), f32(adst)
        Wm = W.reshape(H, H)
        Ws = np.einsum("dhk,hk->dh", W.reshape(H, 4, 16), asrc)
        Wd = np.einsum("dhk,hk->dh", W.reshape(H, 4, 16), adst)
        return np.ascontiguousarray(np.concatenate([Wm, Ws, Wd], axis=1))

    Wcat0 = wcat(W0, asrc0, adst0)
    Wcat1 = wcat(W1, asrc1, adst1)
    Wae0 = np.ascontiguousarray(np.einsum("dhk,hk->dh", f32(We0).reshape(EDGE_IN, 4, 16), f32(ae0)))
    Wae1 = np.ascontiguousarray(np.einsum("dhk,hk->dh", f32(We1).reshape(EDGE_IN, 4, 16), f32(ae1)))
    ident = np.eye(128, dtype=np.float32)
    iota_r = np.broadcast_to(np.arange(128, dtype=np.float32)[None, :], (128, 128)).copy()
    tile128 = lambda v: np.ascontiguousarray(np.broadcast_to(f32(v).reshape(1, -1), (128, len(np.ravel(v)))))

    nchb, esrcT, slotT, validT, eaTT = _prep_edges(src, dst, edge_attr)

    # ---- launch A: encoder + layer-0 table ----
    ncA = _cached(("enc",), build_encoder)
    xT = np.ascontiguousarray(x.T)
    maps = [{"xT": np.ascontiguousarray(xT[:, c * NSH:(c + 1) * NSH]),
             "Wenc": W_enc, "benc": tile128(b_enc), "Wcat": Wcat0, "ident": ident}
            for c in range(NC)]
    resA = _run_spmd(ncA, maps)
    tab0 = np.concatenate([resA[c]["tab_out"] for c in range(NC)], axis=0)

    # ---- launch B1: GAT layer 0 -> layer-1 table ----
    ncB1 = _cached(("gat", "table", tuple(nchb)), lambda: build_gat(nchb, "table"))
    zoff = np.zeros((1, 1), np.int32)
    maps = [{"tab": tab0, "esrcT": esrcT[c], "slotT": slotT[c], "validT": validT[c],
             "eaTT": eaTT[c], "Wae": Wae0, "bgat": tile128(b0), "iota": iota_r,
             "ident": ident, "core_off": zoff, "Wcat": Wcat1,
             "aldbase": np.ascontiguousarray(tab0[c * NSH:(c + 1) * NSH, 68:72])}
            for c in range(NC)]
    resB1 = _run_spmd(ncB1, maps)
    tab1 = np.concatenate([resB1[c]["tabn"] for c in range(NC)], axis=0)

    # ---- launch B2: GAT layer 1 -> h2 + pressure ----
    ncB2 = _cached(("gat", "press", float(np.ravel(bp2)[0]), tuple(nchb)),
                   lambda: build_gat(nchb, "press", bp2=float(np.ravel(bp2)[0])))
    maps = [{"tab": tab1, "esrcT": esrcT[c], "slotT": slotT[c], "validT": validT[c],
             "eaTT": eaTT[c], "Wae": Wae1, "bgat": tile128(b1), "iota": iota_r,
             "ident": ident, "core_off": zoff,
             "Wp1": Wp1, "Wp2": Wp2.reshape(H, 1), "bp1": tile128(bp1),
             "aldbase": np.ascontiguousarray(tab1[c * NSH:(c + 1) * NSH, 68:72])}
            for c in range(NC)]
    resB2 = _run_spmd(ncB2, maps)
    h2 = np.concatenate([resB2[c]["h_out"] for c in range(NC)], axis=0)
    pressure = np.concatenate([resB2[c]["press"] for c in range(NC)], axis=0)[:, 0]

    # ---- launch C: flow MLP on original edges ----
    oidx = np.nonzero(np.asarray(is_original_edge))[0][:E_ORIG]
    ncC = _cached(("flow", float(np.ravel(bf2)[0])),
                  lambda: build_flow(bf2=float(np.ravel(bf2)[0])))
    Wf1s = np.ascontiguousarray(Wf1[0:H, :])
    Wf1d = np.ascontiguousarray(Wf1[H:2 * H, :])
    Wf1e = np.ascontiguousarray(Wf1[2 * H:, :])
    maps = []
    for c in range(NC):
        oi = oidx[c * EO_SH:(c + 1) * EO_SH]
        es = np.zeros((NCHO * 128,), np.int32)
        ed = np.zeros((NCHO * 128,), np.int32)
        es[:EO_SH] = src[oi]
        ed[:EO_SH] = dst[oi]
        eao = np.zeros((NCHO * 128, EDGE_IN), np.float32)
        eao[:EO_SH] = edge_attr[oi]
        maps.append({"htab": h2,
                     "esrcT": np.ascontiguousarray(es.reshape(NCHO, 128).T),
                     "edstT": np.ascontiguousarray(ed.reshape(NCHO, 128).T),
                     "eaTT": np.ascontiguousarray(eao.T),
                     "Wf1s": Wf1s, "Wf1d": Wf1d, "Wf1e": Wf1e,
                     "Wf2": Wf2.reshape(H, 1), "bf1": tile128(bf1), "ident": ident})
    resC = _run_spmd(ncC, maps)
    flow = np.concatenate(
        [resC[c]["flowT"].T.ravel()[:EO_SH] for c in range(NC)])

    return pressure.astype(np.float32), flow.astype(np.float32), h2.astype(np.float32)
